# revision 1
# baseline (speedup 1.0000x reference)
"""Trainium2 Bass kernel for ContextualLoss.

Contract: kernel(**inputs) takes FULL inputs {"inputs": [8,128,64,64] f32,
"targets": [8,128,64,64] f32} and returns the FULL scalar loss (np.float32).

Sharding: data-parallel over batch B=8 across the 8 NeuronCores (core b gets
batch element b). The only cross-batch quantities are the target channel mean
y_mu (a [128] broadcast computed on host during input sharding) and the final
mean of the 8 per-batch scalar losses (computed on host during gather).

Per-core math (x, y: [C=128, N=4096], mu: [128,1]):
    xc = x - mu ; yc = y - mu                        (bf16)
    u[n] = 1/max(||xc[:,n]||, 1e-12) ; v likewise for yc
    G = xc^T @ yc                                    (PE, [N,N] in 128-row blocks)
    Sv = G * v[m]   (v folded via a broadcast row);  smax_cos = u * max_m Sv
    a = 1/(1 - smax_cos + eps); sc = a/h
    w = exp(sc*u*Sv + (1/h - sc))  == exp((1 - dist/(dist_min+eps))/h)
    r[n] = sum_m w   (fused accumulator of the Exp pass)
    colmax[m] = max_n w/r  (fused scalar_tensor_tensor, running over blocks)
    loss_b = -log(mean_m colmax + eps)
"""

import numpy as np

import concourse.bass as bass
import concourse.tile as tile
from concourse import bacc, masks, mybir
from concourse.bass_utils import run_bass_kernel_spmd

F32 = mybir.dt.float32
BF16 = mybir.dt.bfloat16
AF = mybir.ActivationFunctionType
OP = mybir.AluOpType

B, C, H, W = 8, 128, 64, 64
N = H * W                  # 4096
P = 128                    # partitions / channels
NBLK = N // P              # 32 row blocks
MM_N = 512                 # matmul moving free dim (one PSUM bank)
MC = 2048                  # m-chunk per PSUM tile (4 banks), 2 chunks/block
H_BW = 0.5
EPS = 1e-5
NORM_EPS = 1e-12
NEG_INF = -3.0e38
N_CORES = 8


def _norm_chain(nc, pool, ssq_ps, name):
    """[128, NBLK] sum-of-squares in PSUM -> inv-norm (f32, SBUF)."""
    nrm = pool.tile([P, NBLK], F32, name=f"nrm_{name}")
    nc.scalar.activation(nrm[:], ssq_ps[:], AF.Sqrt)
    ncl = pool.tile([P, NBLK], F32, name=f"ncl_{name}")
    nc.vector.tensor_scalar_max(ncl[:], nrm[:], NORM_EPS)
    inv = pool.tile([P, NBLK], F32, name=f"inv_{name}")
    nc.vector.reciprocal(inv[:], ncl[:])
    return inv


def _kernel_body(tc):
    nc = tc.nc
    x_d = nc.dram_tensor("x", [P, N], F32, kind="ExternalInput").ap()
    y_d = nc.dram_tensor("y", [P, N], F32, kind="ExternalInput").ap()
    mu_d = nc.dram_tensor("mu", [P, 1], F32, kind="ExternalInput").ap()
    id_d = nc.dram_tensor("ident", [P, P], F32, kind="ExternalInput").ap()
    loss_d = nc.dram_tensor("loss", [1, 1], F32, kind="ExternalOutput").ap()

    from contextlib import ExitStack
    with ExitStack() as ctx:
        persist = ctx.enter_context(tc.tile_pool(name="persist", bufs=1))
        small = ctx.enter_context(tc.tile_pool(name="small", bufs=4))

        # constants
        ident_f = persist.tile([P, P], F32)
        nc.sync.dma_start(ident_f[:], id_d)
        ident_bf = persist.tile([P, P], BF16)
        nc.vector.tensor_copy(ident_bf[:], ident_f[:])
        ones_sq = persist.tile([P, P], BF16)
        nc.vector.memset(ones_sq[:], 1.0)
        ones_col_bf = persist.tile([P, 1], BF16)
        nc.vector.memset(ones_col_bf[:], 1.0)
        ones_col_f = persist.tile([P, 1], F32)
        nc.vector.memset(ones_col_f[:], 1.0)
        # bias constants for ACT (bias APs must be pre-materialized in SBUF)
        c_hbias = persist.tile([P, 1], F32)
        nc.vector.memset(c_hbias[:], H_BW * (1.0 + EPS))
        c_invh = persist.tile([P, 1], F32)
        nc.vector.memset(c_invh[:], 1.0 / H_BW)
        c_eps = persist.tile([P, 1], F32)
        nc.vector.memset(c_eps[:], EPS)

        # ---------- load + center ----------
        xc = persist.tile([P, N], BF16)   # centered x, bf16
        yc = persist.tile([P, N], BF16)   # centered y, bf16
        with tc.tile_pool(name="load", bufs=1) as load:
            mu_sb = persist.tile([P, 1], F32)
            nc.sync.dma_start(mu_sb[:], mu_d)
            x_sb = load.tile([P, N], F32)
            y_sb = load.tile([P, N], F32)
            nc.sync.dma_start(x_sb[:], x_d)
            nc.sync.dma_start(y_sb[:], y_d)
            nc.vector.tensor_scalar_sub(xc[:], x_sb[:], mu_sb[:, 0:1])
            nc.vector.tensor_scalar_sub(yc[:], y_sb[:], mu_sb[:, 0:1])

            # squares for the channel norms (stay in this pool's scope)
            xsq = load.tile([P, N], BF16)
            nc.scalar.activation(xsq[:], xc[:], AF.Square)
            ysq = load.tile([P, N], BF16)
            nc.scalar.activation(ysq[:], yc[:], AF.Square)

            # ssq in [n_lo(partitions), n_hi] layout: 1 matmul per 128-col chunk
            with tc.tile_pool(name="ssq_ps", bufs=1, space="PSUM") as sp:
                ssq_x = sp.tile([P, NBLK], F32)
                ssq_y = sp.tile([P, NBLK], F32)
                for j in range(NBLK):
                    nc.tensor.matmul(ssq_x[:, j:j + 1], xsq[:, j * P:(j + 1) * P],
                                     ones_col_bf[:], start=True, stop=True)
                for j in range(NBLK):
                    nc.tensor.matmul(ssq_y[:, j:j + 1], ysq[:, j * P:(j + 1) * P],
                                     ones_col_bf[:], start=True, stop=True)
                u_col = _norm_chain(nc, persist, ssq_x, "x")    # [128, 32] f32
                v_col = _norm_chain(nc, persist, ssq_y, "y")    # [128, 32] f32
            hu_col = persist.tile([P, NBLK], F32)               # -h * u
            nc.vector.tensor_scalar_mul(hu_col[:], u_col[:], -H_BW)

        # ---------- broadcast v across partitions: vrow[p, m] = v[m] ----------
        vrow = persist.tile([P, N], BF16)
        with tc.tile_pool(name="vrow_ps_pool", bufs=1, space="PSUM") as vp, \
             tc.tile_pool(name="diag_pool", bufs=1) as dp:
            vrow_ps = vp.tile([P, N], F32)
            diag_all = dp.tile([P, N], BF16)
            for j in range(NBLK):
                # diag_j = identity * v_col[:, j]  (per-partition scalar)
                nc.vector.tensor_scalar_mul(diag_all[:, j * P:(j + 1) * P],
                                            ident_bf[:], v_col[:, j:j + 1])
            for j in range(NBLK):
                # ones^T @ diag_j -> each row p gets v[128j : 128j+128]
                nc.tensor.matmul(vrow_ps[:, j * P:(j + 1) * P], ones_sq[:],
                                 diag_all[:, j * P:(j + 1) * P],
                                 start=True, stop=True)
            nc.scalar.activation(vrow[:], vrow_ps[:], AF.Copy)

        # ---------- main loop over 32 row blocks ----------
        colmax = persist.tile([P, N], BF16)
        nc.vector.memset(colmax[:], 0.0)

        # normalize-split: DVE multiplies cols [0, DSPLIT), ACT the rest
        DSPLIT = 512
        with tc.tile_pool(name="g_ps_pool", bufs=2, space="PSUM") as gp, \
             tc.tile_pool(name="sv_pool", bufs=3) as svp, \
             tc.tile_pool(name="w_pool", bufs=3) as wp, \
             tc.tile_pool(name="wn_pool", bufs=3) as wnp:
            for nb in range(NBLK):
                lhsT = xc[:, nb * P:(nb + 1) * P]
                sv = svp.tile([P, N], BF16)
                gv = small.tile([P, 1], F32, name="gv")
                gvp = small.tile([P, N // MC], F32, name="gvp")
                for ch in range(N // MC):
                    g_ps = gp.tile([P, MC], F32, name="g_ps")
                    for j in range(MC // MM_N):
                        m0 = ch * MC + j * MM_N
                        nc.tensor.matmul(g_ps[:, j * MM_N:(j + 1) * MM_N],
                                         lhsT, yc[:, m0:m0 + MM_N],
                                         start=True, stop=True)
                    # Sv = G * vrow (bf16); row-max per chunk (vanilla ISA)
                    nc.vector.tensor_mul(sv[:, ch * MC:(ch + 1) * MC],
                                         g_ps[:],
                                         vrow[:, ch * MC:(ch + 1) * MC])
                    nc.vector.reduce_max(
                        gvp[:, ch:ch + 1],
                        sv[:, ch * MC:(ch + 1) * MC],
                        axis=mybir.AxisListType.X)

                nc.vector.reduce_max(gv[:], gvp[:], axis=mybir.AxisListType.X)
                # t = h*(1+eps) - h*u*gvmax ; sc = 1/t ; scale_eff = sc*u ;
                # bias_v = 1/h - sc        (all tiny per-partition DVE ops)
                t = small.tile([P, 1], F32, name="t")
                nc.vector.tensor_scalar(t[:], gv[:], hu_col[:, nb:nb + 1],
                                        c_hbias[:, 0:1], OP.mult, OP.add)
                sc = small.tile([P, 1], F32, name="sc")
                nc.vector.reciprocal(sc[:], t[:])
                scale_eff = small.tile([P, 1], F32, name="scale_eff")
                nc.vector.tensor_scalar_mul(scale_eff[:], sc[:],
                                            u_col[:, nb:nb + 1])
                bias_v = small.tile([P, 1], F32, name="bias_v")
                nc.vector.tensor_scalar(bias_v[:], sc[:], -1.0,
                                        c_invh[:, 0:1], OP.mult, OP.add)

                # w = exp(scale_eff*Sv + bias_v); r = rowsum(w) fused
                w = wp.tile([P, N], BF16)
                r = small.tile([P, 1], F32, name="r")
                nc.scalar.activation(w[:], sv[:], AF.Exp,
                                     bias=bias_v[:, 0:1],
                                     scale=scale_eff[:, 0:1],
                                     accum_out=r[:, 0:1])
                rinv = small.tile([P, 1], F32, name="rinv")
                nc.vector.reciprocal(rinv[:], r[:])
                # wn = w * rinv: DVE takes [0,DSPLIT) @4x, ACT the rest
                wn = wnp.tile([P, N], BF16)
                nc.vector.tensor_scalar_mul(wn[:, 0:DSPLIT], w[:, 0:DSPLIT],
                                            rinv[:, 0:1])
                nc.scalar.activation(wn[:, DSPLIT:N], w[:, DSPLIT:N],
                                     AF.Identity, bias=0.0,
                                     scale=rinv[:, 0:1])
                # colmax = max(colmax, wn)  @2x
                nc.vector.tensor_tensor(colmax[:], wn[:], colmax[:], OP.max)

        # ---------- tail: partition-max via PE transpose, then mean+log ----------
        # all-f32 tail: bf16-in-PSUM packing diverges between sim and HW
        cm_f = persist.tile([P, N], F32)
        nc.vector.tensor_copy(cm_f[:], colmax[:])
        cm_col = persist.tile([P, NBLK], F32)
        with tc.tile_pool(name="tail_ps", bufs=2, space="PSUM") as tp:
            HB = NBLK // 2
            for k in range(2):
                cmT = tp.tile([P, N // 2], F32, name="cmT")
                for j in range(HB):
                    nc.tensor.transpose(cmT[:, j * P:(j + 1) * P],
                                        cm_f[:, (k * HB + j) * P:(k * HB + j + 1) * P],
                                        ident_f[:])
                nc.vector.reduce_max(cm_col[:, k * HB:(k + 1) * HB],
                                     cmT[:].rearrange("p (j q) -> p j q", q=P),
                                     axis=mybir.AxisListType.X)
        cm_sum = persist.tile([P, 1], F32)
        nc.vector.reduce_sum(cm_sum[:], cm_col[:], axis=mybir.AxisListType.X)
        with tc.tile_pool(name="tot_ps", bufs=1, space="PSUM") as tp2:
            total = tp2.tile([1, 1], F32)
            nc.tensor.matmul(total[:], cm_sum[:], ones_col_f[:],
                             start=True, stop=True)
            lnv = persist.tile([1, 1], F32)
            nc.scalar.activation(lnv[:], total[:], AF.Ln,
                                 bias=c_eps[0:1, 0:1], scale=1.0 / N)
            loss_sb = persist.tile([1, 1], F32)
            nc.vector.tensor_scalar_mul(loss_sb[:], lnv[:], -1.0)
            nc.sync.dma_start(loss_d, loss_sb[:])


_NC_CACHE = None


def _get_nc():
    global _NC_CACHE
    if _NC_CACHE is None:
        nc = bacc.Bacc("TRN2", target_bir_lowering=False, debug=False)
        with tile.TileContext(nc) as tc:
            _kernel_body(tc)
        nc.compile()
        _NC_CACHE = nc
    return _NC_CACHE


def kernel(inputs, targets):
    x = np.ascontiguousarray(np.asarray(inputs, dtype=np.float32))
    y = np.ascontiguousarray(np.asarray(targets, dtype=np.float32))
    assert x.shape == (B, C, H, W) and y.shape == (B, C, H, W)
    mu = y.mean(axis=(0, 2, 3)).astype(np.float32).reshape(C, 1)
    in_maps = [
        {
            "x": x[b].reshape(C, N),
            "y": y[b].reshape(C, N),
            "mu": mu,
            "ident": np.eye(P, dtype=np.float32),
        }
        for b in range(B)
    ]
    nc = _get_nc()
    res = run_bass_kernel_spmd(nc, in_maps, list(range(N_CORES)))
    losses = [float(res.results[b]["loss"][0, 0]) for b in range(B)]
    return np.float32(np.mean(losses))



# revision 12
# speedup vs baseline: 1.0726x; 1.0726x over previous
"""Trainium2 Bass kernel for ContextualLoss.

Contract: kernel(**inputs) takes FULL inputs {"inputs": [8,128,64,64] f32,
"targets": [8,128,64,64] f32} and returns the FULL scalar loss (np.float32).

Sharding: data-parallel over batch B=8 across the 8 NeuronCores (core b gets
batch element b). The only cross-batch quantities are the target channel mean
y_mu (a [128] broadcast computed on host during input sharding) and the final
mean of the 8 per-batch scalar losses (computed on host during gather).

Per-core math (x, y: [C=128, N=4096], mu: [128,1]):
    xc = x - mu ; yc = y - mu                        (bf16)
    u[n] = 1/max(||xc[:,n]||, 1e-12) ; v likewise for yc
    G = xc^T @ yc                                    (PE, [N,N] in 128-row blocks)
    TTR (one DVE pass): sv = G * vrow  AND  gv[n] = max_m sv  (accum chained
        across the two 2048-col PSUM chunks)
    a = 1/(1 - u*gv + eps); sc = a/h  (tiny per-partition ops on ACT + DVE)
    w = exp(sc*u*sv + (1/h - sc)) == exp((1 - dist/(dist_min+eps))/h)  [ACT,
        fused rowsum accumulator r]
    colmax[m] = max_n w/r: fused scalar_tensor_tensor, split between DVE
        (cols [0, DX)) and GPSIMD (cols [DX, N)) to balance engine load.
    loss_b = -log(mean_m colmax + eps)
"""

import numpy as np

import concourse.bass as bass
import concourse.tile as tile
from concourse import bacc, masks, mybir
from concourse.bass_utils import run_bass_kernel_spmd

F32 = mybir.dt.float32
BF16 = mybir.dt.bfloat16
AF = mybir.ActivationFunctionType
OP = mybir.AluOpType

B, C, H, W = 8, 128, 64, 64
N = H * W                  # 4096
P = 128                    # partitions / channels
NBLK = N // P              # 32 row blocks
MM_N = 512                 # matmul moving free dim (one PSUM bank)
MC = 2048                  # m-chunk per PSUM tile (4 banks), 2 chunks/block
DX = 512                   # colmax cols handled by DVE; rest go to GPSIMD
H_BW = 0.5
EPS = 1e-5
NORM_EPS = 1e-12
NEG_INF = -3.0e38
N_CORES = 8


def _norm_chain(nc, pool, ssq_ps, name):
    """[128, NBLK] sum-of-squares in PSUM -> inv-norm (f32, SBUF)."""
    nrm = pool.tile([P, NBLK], F32, name=f"nrm_{name}")
    nc.scalar.activation(nrm[:], ssq_ps[:], AF.Sqrt)
    ncl = pool.tile([P, NBLK], F32, name=f"ncl_{name}")
    nc.vector.tensor_scalar_max(ncl[:], nrm[:], NORM_EPS)
    inv = pool.tile([P, NBLK], F32, name=f"inv_{name}")
    nc.vector.reciprocal(inv[:], ncl[:])
    return inv


def _kernel_body(tc):
    nc = tc.nc
    x_d = nc.dram_tensor("x", [P, N], F32, kind="ExternalInput").ap()
    y_d = nc.dram_tensor("y", [P, N], F32, kind="ExternalInput").ap()
    mu_d = nc.dram_tensor("mu", [P, 1], F32, kind="ExternalInput").ap()
    id_d = nc.dram_tensor("ident", [P, P], F32, kind="ExternalInput").ap()
    loss_d = nc.dram_tensor("loss", [1, 1], F32, kind="ExternalOutput").ap()

    from contextlib import ExitStack
    with ExitStack() as ctx:
        persist = ctx.enter_context(tc.tile_pool(name="persist", bufs=1))
        small = ctx.enter_context(tc.tile_pool(name="small", bufs=4))

        # constants
        ident_f = persist.tile([P, P], F32)
        nc.sync.dma_start(ident_f[:], id_d)
        ident_bf = persist.tile([P, P], BF16)
        nc.vector.tensor_copy(ident_bf[:], ident_f[:])
        ones_sq = persist.tile([P, P], BF16)
        nc.vector.memset(ones_sq[:], 1.0)
        ones_col_bf = persist.tile([P, 1], BF16)
        nc.vector.memset(ones_col_bf[:], 1.0)
        ones_col_f = persist.tile([P, 1], F32)
        nc.vector.memset(ones_col_f[:], 1.0)
        # bias constants for ACT (bias APs must be pre-materialized in SBUF)
        c_hbias = persist.tile([P, 1], F32)
        nc.vector.memset(c_hbias[:], H_BW * (1.0 + EPS))
        c_invh = persist.tile([P, 1], F32)
        nc.vector.memset(c_invh[:], 1.0 / H_BW)
        c_eps = persist.tile([P, 1], F32)
        nc.vector.memset(c_eps[:], EPS)
        c_zero = persist.tile([P, 1], F32)
        nc.vector.memset(c_zero[:], 0.0)

        # ---------- load + center ----------
        xc = persist.tile([P, N], BF16)   # centered x, bf16
        yc = persist.tile([P, N], BF16)   # centered y, bf16
        with tc.tile_pool(name="load", bufs=1) as load:
            mu_sb = persist.tile([P, 1], F32)
            nc.sync.dma_start(mu_sb[:], mu_d)
            x_sb = load.tile([P, N], F32)
            y_sb = load.tile([P, N], F32)
            nc.sync.dma_start(x_sb[:], x_d)
            nc.sync.dma_start(y_sb[:], y_d)
            nc.vector.tensor_scalar_sub(xc[:], x_sb[:], mu_sb[:, 0:1])
            nc.vector.tensor_scalar_sub(yc[:], y_sb[:], mu_sb[:, 0:1])

            # squares for the channel norms (stay in this pool's scope)
            xsq = load.tile([P, N], BF16)
            nc.scalar.activation(xsq[:], xc[:], AF.Square)
            ysq = load.tile([P, N], BF16)
            nc.scalar.activation(ysq[:], yc[:], AF.Square)

            # ssq in [n_lo(partitions), n_hi] layout: 1 matmul per 128-col chunk
            with tc.tile_pool(name="ssq_ps", bufs=1, space="PSUM") as sp:
                ssq_x = sp.tile([P, NBLK], F32)
                ssq_y = sp.tile([P, NBLK], F32)
                for j in range(NBLK):
                    nc.tensor.matmul(ssq_x[:, j:j + 1], xsq[:, j * P:(j + 1) * P],
                                     ones_col_bf[:], start=True, stop=True)
                for j in range(NBLK):
                    nc.tensor.matmul(ssq_y[:, j:j + 1], ysq[:, j * P:(j + 1) * P],
                                     ones_col_bf[:], start=True, stop=True)
                u_col = _norm_chain(nc, persist, ssq_x, "x")    # [128, 32] f32
                v_col = _norm_chain(nc, persist, ssq_y, "y")    # [128, 32] f32
            hu_col = persist.tile([P, NBLK], F32)               # -h * u
            nc.vector.tensor_scalar_mul(hu_col[:], u_col[:], -H_BW)

        # ---------- broadcast v across partitions: vrow[p, m] = v[m] ----------
        # then fold it into yc: yv[:, m] = yc[:, m] * v[m], so the PE output
        # G' = xc^T @ yv is Sv directly (rowmax + exp both read PSUM).
        yv = persist.tile([P, N], BF16)
        with tc.tile_pool(name="vrow_ps_pool", bufs=1, space="PSUM") as vp, \
             tc.tile_pool(name="diag_pool", bufs=1) as dp:
            vrow_ps = vp.tile([P, N], F32)
            diag_all = dp.tile([P, N], BF16)
            vrow = dp.tile([P, N], BF16)
            for j in range(NBLK):
                # diag_j = identity * v_col[:, j]  (per-partition scalar)
                nc.vector.tensor_scalar_mul(diag_all[:, j * P:(j + 1) * P],
                                            ident_bf[:], v_col[:, j:j + 1])
            for j in range(NBLK):
                # ones^T @ diag_j -> each row p gets v[128j : 128j+128]
                nc.tensor.matmul(vrow_ps[:, j * P:(j + 1) * P], ones_sq[:],
                                 diag_all[:, j * P:(j + 1) * P],
                                 start=True, stop=True)
            nc.scalar.activation(vrow[:], vrow_ps[:], AF.Copy)
            nc.vector.tensor_mul(yv[:], yc[:], vrow[:])

        # ---------- main loop over 32 row blocks ----------
        colmax = persist.tile([P, N], BF16)
        nc.vector.memset(colmax[:], 0.0)

        NCH = N // MC
        with tc.tile_pool(name="g_ps_pool", bufs=2, space="PSUM") as gp, \
             tc.tile_pool(name="w_pool", bufs=3) as wp, \
             tc.tile_pool(name="wn_pool", bufs=3) as wnp:
            for nb in range(NBLK):
                lhsT = xc[:, nb * P:(nb + 1) * P]
                gvp = small.tile([P, NCH], F32, name="gvp")
                gv = small.tile([P, 1], F32, name="gv")
                # ---- pass 1: Sv chunks -> rowmax straight from PSUM ----
                for ch in range(NCH):
                    g_ps = gp.tile([P, MC], F32, name="g_ps")
                    for j in range(MC // MM_N):
                        m0 = ch * MC + j * MM_N
                        nc.tensor.matmul(g_ps[:, j * MM_N:(j + 1) * MM_N],
                                         lhsT, yv[:, m0:m0 + MM_N],
                                         start=True, stop=True)
                    nc.vector.reduce_max(gvp[:, ch:ch + 1], g_ps[:],
                                         axis=mybir.AxisListType.X)
                nc.vector.reduce_max(gv[:], gvp[:], axis=mybir.AxisListType.X)

                # t = h*(1+eps) - h*u*gvmax  (ACT: per-partition scale+bias)
                t = small.tile([P, 1], F32, name="t")
                nc.scalar.activation(t[:], gv[:], AF.Identity,
                                     bias=c_hbias[:, 0:1],
                                     scale=hu_col[:, nb:nb + 1])
                sc = small.tile([P, 1], F32, name="sc")
                nc.vector.reciprocal(sc[:], t[:])
                # scale_eff = sc*u ; bias_v = 1/h - sc   (ACT)
                scale_eff = small.tile([P, 1], F32, name="scale_eff")
                nc.scalar.activation(scale_eff[:], sc[:], AF.Identity,
                                     bias=0.0,
                                     scale=u_col[:, nb:nb + 1])
                bias_v = small.tile([P, 1], F32, name="bias_v")
                nc.scalar.activation(bias_v[:], sc[:], AF.Identity,
                                     bias=c_invh[:, 0:1], scale=-1.0)

                # ---- pass 2: recompute Sv chunks (PE has slack); exp reads
                # PSUM directly -> w (f32, for normalize_recip); r = rowsum ----
                w = wp.tile([P, N], F32)
                rch = small.tile([P, NCH], F32, name="rch")
                r = small.tile([P, 1], F32, name="r")
                for ch in range(NCH):
                    g_ps2 = gp.tile([P, MC], F32, name="g_ps")
                    for j in range(MC // MM_N):
                        m0 = ch * MC + j * MM_N
                        nc.tensor.matmul(g_ps2[:, j * MM_N:(j + 1) * MM_N],
                                         lhsT, yv[:, m0:m0 + MM_N],
                                         start=True, stop=True)
                    nc.scalar.activation(w[:, ch * MC:(ch + 1) * MC], g_ps2[:],
                                         AF.Exp,
                                         bias=bias_v[:, 0:1],
                                         scale=scale_eff[:, 0:1],
                                         accum_out=rch[:, ch:ch + 1])
                nc.vector.reduce_sum(r[:], rch[:], axis=mybir.AxisListType.X)
                # wn = w / r on the otherwise-idle GPSIMD (also forms 1/r)
                wn = wnp.tile([P, N], BF16)
                nc.gpsimd.normalize_recip(wn[:], w[:], r[:, 0:1])
                # colmax = max(colmax, wn)  (DVE tensor_tensor, 2x bf16)
                nc.vector.tensor_tensor(colmax[:], wn[:], colmax[:], OP.max)

        # ---------- tail: partition-max via PE transpose, then mean+log ----------
        # all-f32 tail: bf16-in-PSUM packing diverges between sim and HW
        cm_f = persist.tile([P, N], F32)
        nc.vector.tensor_copy(cm_f[:], colmax[:])
        cm_col = persist.tile([P, NBLK], F32)
        with tc.tile_pool(name="tail_ps", bufs=2, space="PSUM") as tp:
            HB = NBLK // 2
            for k in range(2):
                cmT = tp.tile([P, N // 2], F32, name="cmT")
                for j in range(HB):
                    jj = k * HB + j
                    nc.tensor.transpose(cmT[:, j * P:(j + 1) * P],
                                        cm_f[:, jj * P:(jj + 1) * P],
                                        ident_f[:])
                nc.vector.reduce_max(cm_col[:, k * HB:(k + 1) * HB],
                                     cmT[:].rearrange("p (j q) -> p j q", q=P),
                                     axis=mybir.AxisListType.X)
        cm_sum = persist.tile([P, 1], F32)
        nc.vector.reduce_sum(cm_sum[:], cm_col[:], axis=mybir.AxisListType.X)
        with tc.tile_pool(name="tot_ps", bufs=1, space="PSUM") as tp2:
            total = tp2.tile([1, 1], F32)
            nc.tensor.matmul(total[:], cm_sum[:], ones_col_f[:],
                             start=True, stop=True)
            lnv = persist.tile([1, 1], F32)
            nc.scalar.activation(lnv[:], total[:], AF.Ln,
                                 bias=c_eps[0:1, 0:1], scale=1.0 / N)
            loss_sb = persist.tile([1, 1], F32)
            nc.vector.tensor_scalar_mul(loss_sb[:], lnv[:], -1.0)
            nc.sync.dma_start(loss_d, loss_sb[:])


_NC_CACHE = None


def _get_nc():
    global _NC_CACHE
    if _NC_CACHE is None:
        nc = bacc.Bacc("TRN2", target_bir_lowering=False, debug=False)
        with tile.TileContext(nc) as tc:
            _kernel_body(tc)
        nc.compile()
        _NC_CACHE = nc
    return _NC_CACHE


def kernel(inputs, targets):
    x = np.ascontiguousarray(np.asarray(inputs, dtype=np.float32))
    y = np.ascontiguousarray(np.asarray(targets, dtype=np.float32))
    assert x.shape == (B, C, H, W) and y.shape == (B, C, H, W)
    mu = y.mean(axis=(0, 2, 3)).astype(np.float32).reshape(C, 1)
    in_maps = [
        {
            "x": x[b].reshape(C, N),
            "y": y[b].reshape(C, N),
            "mu": mu,
            "ident": np.eye(P, dtype=np.float32),
        }
        for b in range(B)
    ]
    nc = _get_nc()
    res = run_bass_kernel_spmd(nc, in_maps, list(range(N_CORES)))
    losses = [float(res.results[b]["loss"][0, 0]) for b in range(B)]
    return np.float32(np.mean(losses))


# revision 14
# speedup vs baseline: 1.2215x; 1.1388x over previous
"""Trainium2 Bass kernel for ContextualLoss.

Contract: kernel(**inputs) takes FULL inputs {"inputs": [8,128,64,64] f32,
"targets": [8,128,64,64] f32} and returns the FULL scalar loss (np.float32).

Sharding: data-parallel over batch B=8 across the 8 NeuronCores (core b gets
batch element b). The only cross-batch quantities are the target channel mean
y_mu (a [128] broadcast computed on host during input sharding) and the final
mean of the 8 per-batch scalar losses (computed on host during gather).

Per-core math (x, y: [C=128, N=4096], mu: [128,1]):
    xc = x - mu ; yc = y - mu                        (bf16)
    u[n] = 1/max(||xc[:,n]||, 1e-12) ; v likewise for yc
    G = xc^T @ yc                                    (PE, [N,N] in 128-row blocks)
    TTR (one DVE pass): sv = G * vrow  AND  gv[n] = max_m sv  (accum chained
        across the two 2048-col PSUM chunks)
    a = 1/(1 - u*gv + eps); sc = a/h  (tiny per-partition ops on ACT + DVE)
    w = exp(sc*u*sv + (1/h - sc)) == exp((1 - dist/(dist_min+eps))/h)  [ACT,
        fused rowsum accumulator r]
    colmax[m] = max_n w/r: fused scalar_tensor_tensor, split between DVE
        (cols [0, DX)) and GPSIMD (cols [DX, N)) to balance engine load.
    loss_b = -log(mean_m colmax + eps)
"""

import numpy as np

import concourse.bass as bass
import concourse.tile as tile
from concourse import bacc, masks, mybir
from concourse.bass_utils import run_bass_kernel_spmd

F32 = mybir.dt.float32
BF16 = mybir.dt.bfloat16
AF = mybir.ActivationFunctionType
OP = mybir.AluOpType

B, C, H, W = 8, 128, 64, 64
N = H * W                  # 4096
P = 128                    # partitions / channels
NBLK = N // P              # 32 row blocks
MM_N = 512                 # matmul moving free dim (one PSUM bank)
MC = 2048                  # m-chunk per PSUM tile (4 banks), 2 chunks/block
DX = 512                   # colmax cols handled by DVE; rest go to GPSIMD
H_BW = 0.5
EPS = 1e-5
NORM_EPS = 1e-12
NEG_INF = -3.0e38
N_CORES = 8


def _norm_chain(nc, pool, ssq_ps, name):
    """[128, NBLK] sum-of-squares in PSUM -> inv-norm (f32, SBUF)."""
    nrm = pool.tile([P, NBLK], F32, name=f"nrm_{name}")
    nc.scalar.activation(nrm[:], ssq_ps[:], AF.Sqrt)
    ncl = pool.tile([P, NBLK], F32, name=f"ncl_{name}")
    nc.vector.tensor_scalar_max(ncl[:], nrm[:], NORM_EPS)
    inv = pool.tile([P, NBLK], F32, name=f"inv_{name}")
    nc.vector.reciprocal(inv[:], ncl[:])
    return inv


def _kernel_body(tc):
    nc = tc.nc
    x_d = nc.dram_tensor("x", [P, N], F32, kind="ExternalInput").ap()
    y_d = nc.dram_tensor("y", [P, N], F32, kind="ExternalInput").ap()
    mu_d = nc.dram_tensor("mu", [P, 1], F32, kind="ExternalInput").ap()
    id_d = nc.dram_tensor("ident", [P, P], F32, kind="ExternalInput").ap()
    loss_d = nc.dram_tensor("loss", [1, 1], F32, kind="ExternalOutput").ap()

    from contextlib import ExitStack
    with ExitStack() as ctx:
        persist = ctx.enter_context(tc.tile_pool(name="persist", bufs=1))
        small = ctx.enter_context(tc.tile_pool(name="small", bufs=4))

        # constants
        ident_f = persist.tile([P, P], F32)
        nc.sync.dma_start(ident_f[:], id_d)
        ident_bf = persist.tile([P, P], BF16)
        nc.vector.tensor_copy(ident_bf[:], ident_f[:])
        ones_sq = persist.tile([P, P], BF16)
        nc.vector.memset(ones_sq[:], 1.0)
        ones_col_bf = persist.tile([P, 1], BF16)
        nc.vector.memset(ones_col_bf[:], 1.0)
        ones_col_f = persist.tile([P, 1], F32)
        nc.vector.memset(ones_col_f[:], 1.0)
        # bias constants for ACT (bias APs must be pre-materialized in SBUF)
        c_hbias = persist.tile([P, 1], F32)
        nc.vector.memset(c_hbias[:], H_BW * (1.0 + EPS))
        c_invh = persist.tile([P, 1], F32)
        nc.vector.memset(c_invh[:], 1.0 / H_BW)
        c_eps = persist.tile([P, 1], F32)
        nc.vector.memset(c_eps[:], EPS)
        c_zero = persist.tile([P, 1], F32)
        nc.vector.memset(c_zero[:], 0.0)

        # ---------- load + center ----------
        xc = persist.tile([P, N], BF16)   # centered x, bf16
        yc = persist.tile([P, N], BF16)   # centered y, bf16
        with tc.tile_pool(name="load", bufs=1) as load:
            mu_sb = persist.tile([P, 1], F32)
            nc.sync.dma_start(mu_sb[:], mu_d)
            x_sb = load.tile([P, N], F32)
            y_sb = load.tile([P, N], F32)
            nc.sync.dma_start(x_sb[:], x_d)
            nc.sync.dma_start(y_sb[:], y_d)
            nc.vector.tensor_scalar_sub(xc[:], x_sb[:], mu_sb[:, 0:1])
            nc.vector.tensor_scalar_sub(yc[:], y_sb[:], mu_sb[:, 0:1])

            # squares for the channel norms (stay in this pool's scope)
            xsq = load.tile([P, N], BF16)
            nc.scalar.activation(xsq[:], xc[:], AF.Square)
            ysq = load.tile([P, N], BF16)
            nc.scalar.activation(ysq[:], yc[:], AF.Square)

            # ssq in [n_lo(partitions), n_hi] layout: 1 matmul per 128-col chunk
            with tc.tile_pool(name="ssq_ps", bufs=1, space="PSUM") as sp:
                ssq_x = sp.tile([P, NBLK], F32)
                ssq_y = sp.tile([P, NBLK], F32)
                for j in range(NBLK):
                    nc.tensor.matmul(ssq_x[:, j:j + 1], xsq[:, j * P:(j + 1) * P],
                                     ones_col_bf[:], start=True, stop=True)
                for j in range(NBLK):
                    nc.tensor.matmul(ssq_y[:, j:j + 1], ysq[:, j * P:(j + 1) * P],
                                     ones_col_bf[:], start=True, stop=True)
                u_col = _norm_chain(nc, persist, ssq_x, "x")    # [128, 32] f32
                v_col = _norm_chain(nc, persist, ssq_y, "y")    # [128, 32] f32
            hu_col = persist.tile([P, NBLK], F32)               # -h * u
            nc.vector.tensor_scalar_mul(hu_col[:], u_col[:], -H_BW)

        # ---------- broadcast v across partitions: vrow[p, m] = v[m] ----------
        # then fold it into yc: yv[:, m] = yc[:, m] * v[m], so the PE output
        # G' = xc^T @ yv is Sv directly (rowmax + exp both read PSUM).
        yv = persist.tile([P, N], BF16)
        with tc.tile_pool(name="vrow_ps_pool", bufs=1, space="PSUM") as vp, \
             tc.tile_pool(name="diag_pool", bufs=1) as dp:
            vrow_ps = vp.tile([P, N], F32)
            diag_all = dp.tile([P, N], BF16)
            vrow = dp.tile([P, N], BF16)
            for j in range(NBLK):
                # diag_j = identity * v_col[:, j]  (per-partition scalar)
                nc.vector.tensor_scalar_mul(diag_all[:, j * P:(j + 1) * P],
                                            ident_bf[:], v_col[:, j:j + 1])
            for j in range(NBLK):
                # ones^T @ diag_j -> each row p gets v[128j : 128j+128]
                nc.tensor.matmul(vrow_ps[:, j * P:(j + 1) * P], ones_sq[:],
                                 diag_all[:, j * P:(j + 1) * P],
                                 start=True, stop=True)
            nc.scalar.activation(vrow[:], vrow_ps[:], AF.Copy)
            nc.vector.tensor_mul(yv[:], yc[:], vrow[:])

        # ---------- main loop over 32 row blocks ----------
        colmax = persist.tile([P, N], BF16)
        nc.vector.memset(colmax[:], 0.0)

        MC1 = 1024             # pass-1 chunk (2 banks; pool of 2 -> 4 banks)
        MC2 = 1024             # pass-2 chunk (2 banks; pool of 2 -> 4 banks)
        NCH1 = N // MC1
        NCH2 = N // MC2
        with tc.tile_pool(name="g1_pool", bufs=2, space="PSUM") as gp1, \
             tc.tile_pool(name="g2_pool", bufs=2, space="PSUM") as gp2, \
             tc.tile_pool(name="w_pool", bufs=3) as wp, \
             tc.tile_pool(name="wn_pool", bufs=3) as wnp:
            for nb in range(NBLK):
                lhsT = xc[:, nb * P:(nb + 1) * P]
                gvp = small.tile([P, NCH1], F32, name="gvp")
                gv = small.tile([P, 1], F32, name="gv")
                # ---- pass 1: Sv chunks -> rowmax straight from PSUM ----
                for ch in range(NCH1):
                    g_ps = gp1.tile([P, MC1], F32, name="g_ps")
                    for j in range(MC1 // MM_N):
                        m0 = ch * MC1 + j * MM_N
                        nc.tensor.matmul(g_ps[:, j * MM_N:(j + 1) * MM_N],
                                         lhsT, yv[:, m0:m0 + MM_N],
                                         start=True, stop=True)
                    nc.vector.reduce_max(gvp[:, ch:ch + 1], g_ps[:],
                                         axis=mybir.AxisListType.X)
                nc.vector.reduce_max(gv[:], gvp[:], axis=mybir.AxisListType.X)

                # t = h*(1+eps) - h*u*gvmax  (ACT: per-partition scale+bias)
                t = small.tile([P, 1], F32, name="t")
                nc.scalar.activation(t[:], gv[:], AF.Identity,
                                     bias=c_hbias[:, 0:1],
                                     scale=hu_col[:, nb:nb + 1])
                sc = small.tile([P, 1], F32, name="sc")
                nc.vector.reciprocal(sc[:], t[:])
                # scale_eff = sc*u ; bias_v = 1/h - sc   (ACT)
                scale_eff = small.tile([P, 1], F32, name="scale_eff")
                nc.scalar.activation(scale_eff[:], sc[:], AF.Identity,
                                     bias=0.0,
                                     scale=u_col[:, nb:nb + 1])
                bias_v = small.tile([P, 1], F32, name="bias_v")
                nc.scalar.activation(bias_v[:], sc[:], AF.Identity,
                                     bias=c_invh[:, 0:1], scale=-1.0)

                # ---- pass 2: recompute Sv chunks (PE has slack); exp reads
                # PSUM directly -> w (f32, for normalize_recip); r = rowsum ----
                w = wp.tile([P, N], F32)
                rch = small.tile([P, NCH2], F32, name="rch")
                r = small.tile([P, 1], F32, name="r")
                for ch in range(NCH2):
                    g_ps2 = gp2.tile([P, MC2], F32, name="g_ps2")
                    for j in range(MC2 // MM_N):
                        m0 = ch * MC2 + j * MM_N
                        nc.tensor.matmul(g_ps2[:, j * MM_N:(j + 1) * MM_N],
                                         lhsT, yv[:, m0:m0 + MM_N],
                                         start=True, stop=True)
                    nc.scalar.activation(w[:, ch * MC2:(ch + 1) * MC2], g_ps2[:],
                                         AF.Exp,
                                         bias=bias_v[:, 0:1],
                                         scale=scale_eff[:, 0:1],
                                         accum_out=rch[:, ch:ch + 1])
                nc.vector.reduce_sum(r[:], rch[:], axis=mybir.AxisListType.X)
                # wn = w / r on the otherwise-idle GPSIMD (also forms 1/r)
                wn = wnp.tile([P, N], BF16)
                nc.gpsimd.normalize_recip(wn[:], w[:], r[:, 0:1])
                # colmax = max(colmax, wn)  (DVE tensor_tensor, 2x bf16)
                nc.vector.tensor_tensor(colmax[:], wn[:], colmax[:], OP.max)

        # ---------- tail: partition-max via PE transpose, then mean+log ----------
        # all-f32 tail: bf16-in-PSUM packing diverges between sim and HW
        cm_f = persist.tile([P, N], F32)
        nc.vector.tensor_copy(cm_f[:], colmax[:])
        cm_col = persist.tile([P, NBLK], F32)
        with tc.tile_pool(name="tail_ps", bufs=2, space="PSUM") as tp:
            HB = NBLK // 2
            for k in range(2):
                cmT = tp.tile([P, N // 2], F32, name="cmT")
                for j in range(HB):
                    jj = k * HB + j
                    nc.tensor.transpose(cmT[:, j * P:(j + 1) * P],
                                        cm_f[:, jj * P:(jj + 1) * P],
                                        ident_f[:])
                nc.vector.reduce_max(cm_col[:, k * HB:(k + 1) * HB],
                                     cmT[:].rearrange("p (j q) -> p j q", q=P),
                                     axis=mybir.AxisListType.X)
        cm_sum = persist.tile([P, 1], F32)
        nc.vector.reduce_sum(cm_sum[:], cm_col[:], axis=mybir.AxisListType.X)
        with tc.tile_pool(name="tot_ps", bufs=1, space="PSUM") as tp2:
            total = tp2.tile([1, 1], F32)
            nc.tensor.matmul(total[:], cm_sum[:], ones_col_f[:],
                             start=True, stop=True)
            lnv = persist.tile([1, 1], F32)
            nc.scalar.activation(lnv[:], total[:], AF.Ln,
                                 bias=c_eps[0:1, 0:1], scale=1.0 / N)
            loss_sb = persist.tile([1, 1], F32)
            nc.vector.tensor_scalar_mul(loss_sb[:], lnv[:], -1.0)
            nc.sync.dma_start(loss_d, loss_sb[:])


_NC_CACHE = None


def _get_nc():
    global _NC_CACHE
    if _NC_CACHE is None:
        nc = bacc.Bacc("TRN2", target_bir_lowering=False, debug=False)
        with tile.TileContext(nc) as tc:
            _kernel_body(tc)
        nc.compile()
        _NC_CACHE = nc
    return _NC_CACHE


def kernel(inputs, targets):
    x = np.ascontiguousarray(np.asarray(inputs, dtype=np.float32))
    y = np.ascontiguousarray(np.asarray(targets, dtype=np.float32))
    assert x.shape == (B, C, H, W) and y.shape == (B, C, H, W)
    mu = y.mean(axis=(0, 2, 3)).astype(np.float32).reshape(C, 1)
    in_maps = [
        {
            "x": x[b].reshape(C, N),
            "y": y[b].reshape(C, N),
            "mu": mu,
            "ident": np.eye(P, dtype=np.float32),
        }
        for b in range(B)
    ]
    nc = _get_nc()
    res = run_bass_kernel_spmd(nc, in_maps, list(range(N_CORES)))
    losses = [float(res.results[b]["loss"][0, 0]) for b in range(B)]
    return np.float32(np.mean(losses))
